# revision 20
# baseline (speedup 1.0000x reference)
"""Trainium2 Bass kernel for 16-head self-attention (B=1, T=2048, d=1024).

Sharding: 2 heads per NeuronCore (tensor-parallel over QKV columns / proj
rows) across 8 cores; each core emits a partial [T, d] projection output,
summed on the host.

Device-side dataflow (per core, heads h0/h1 on partition halves):
  qT/kT [128, T] bf16 (head-dim-major) from full-C chunked matmuls
  vT [128, T] f32 likewise, PE-transposed into per-k-tile [t, v|1] bf16 tiles
  scores S[k, q] per k-tile via row-tiled matmul pairs (both heads share
  the PE array concurrently); exp on ACT -> bf16 e-tiles [128, 2, 512]
  y[65, q] accumulated per k-tile (row 64 = softmax sums via ones column)
  y normalized pre-projection (fast reciprocal + partition_broadcast + STT)
  proj with C=128 (both heads contracted in one matmul); f32 out DMA
K/V/Q generation and the previous group's projection are interleaved into
the attention loop, and scores run two k-tile pairs ahead of the exp so
the scalar engine (exp) stays the pacing engine.
"""
import sys, os

sys.path.insert(0, "/opt/trn_rl_repo")

import numpy as np
import ml_dtypes

import concourse.bass as bass
import concourse.bacc as bacc
import concourse.tile as tile
from concourse import mybir
from concourse import bass_utils
from concourse.masks import make_identity

BF16 = mybir.dt.bfloat16
F32 = mybir.dt.float32

B, T, D = 1, 2048, 1024
H = 16
HD = D // H          # 64
NCORES = 8
HPD = H // NCORES    # 2 heads per device
DL = HPD * HD        # 128 local head dims per device
QG = 4               # q groups of 512
QGS = T // QG        # 512
KT = T // 128        # 16 k tiles
PAIRS = KT // 2      # 8 k-tile pairs per q group

last_results = None  # BassKernelResults of the most recent run (for test.py)

Exp = mybir.ActivationFunctionType.Exp
MULT = mybir.AluOpType.mult
DIV = mybir.AluOpType.divide


def build_nc():
    nc = bacc.Bacc("TRN2", target_bir_lowering=False, debug=False,
                   num_devices=NCORES)
    xT = nc.dram_tensor("xT", [D, T], BF16, kind="ExternalInput").ap()
    wq = nc.dram_tensor("wq", [D, DL], BF16, kind="ExternalInput").ap()
    wk = nc.dram_tensor("wk", [D, DL], BF16, kind="ExternalInput").ap()
    wv = nc.dram_tensor("wv", [D, DL], BF16, kind="ExternalInput").ap()
    wp = nc.dram_tensor("wp", [DL, D], BF16, kind="ExternalInput").ap()
    out = nc.dram_tensor("out", [T, D], F32, kind="ExternalOutput").ap()

    with tile.TileContext(nc) as tc:
        with (
            tc.tile_pool(name="const", bufs=1) as cpool,
            tc.tile_pool(name="work", bufs=2) as spool,
            tc.tile_pool(name="epool", bufs=6) as epool,
            tc.tile_pool(name="opool", bufs=2) as opool,
            tc.tile_pool(name="mm", bufs=2, space="PSUM") as mm_psum,
            tc.tile_pool(name="sc", bufs=2, space="PSUM") as sc_psum,
            tc.tile_pool(name="yp", bufs=2, space="PSUM") as y_psum,
        ):
            # ---- persistent SBUF tensors ----
            xT_sb = cpool.tile([128, 8, T], BF16)       # x^T, d-chunk-major
            wq_sb = cpool.tile([128, 8, DL], BF16)
            wk_sb = cpool.tile([128, 8, DL], BF16)
            wv_sb = cpool.tile([128, 8, DL], BF16)
            wp_sb = cpool.tile([128, D], BF16)          # both heads' proj rows
            ident = cpool.tile([128, 128], F32)
            qT_sb = cpool.tile([128, T], BF16)
            kT_sb = cpool.tile([128, T], BF16)
            vT_sb = cpool.tile([128, T], F32)           # v^T (pre-transpose)
            v_sb = cpool.tile([128, KT, 130], BF16)     # [t, kt, v0|1|v1|1]
            yT_sb = cpool.tile([128, T], BF16)          # normalized y^T

            # ---- input DMAs ----
            # weights as single batched transfers; xT chunked along d AND
            # halved along t so group-0 K/Q matmuls can start early
            xTr = xT.rearrange("(n p) t -> p n t", p=128)
            wqr = wq.rearrange("(n p) m -> p n m", p=128)
            wkr = wk.rearrange("(n p) m -> p n m", p=128)
            wvr = wv.rearrange("(n p) m -> p n m", p=128)
            for kk in range(8):
                eng = nc.sync if kk % 2 == 0 else nc.gpsimd
                eng.dma_start(wk_sb[:, kk, :], wkr[:, kk, :])
                eng.dma_start(wq_sb[:, kk, :], wqr[:, kk, :])
            for kk in range(8):
                eng = nc.sync if kk % 2 == 0 else nc.gpsimd
                eng.dma_start(xT_sb[:, kk, 0:1024], xTr[:, kk, 0:1024])
            for kk in range(8):
                eng = nc.sync if kk % 2 == 0 else nc.gpsimd
                eng.dma_start(wv_sb[:, kk, :], wvr[:, kk, :])
            nc.gpsimd.dma_start(wp_sb[:], wp[:, :])
            for kk in range(8):
                eng = nc.sync if kk % 2 == 0 else nc.gpsimd
                eng.dma_start(xT_sb[:, kk, 1024:2048], xTr[:, kk, 1024:2048])

            make_identity(nc, ident[:])
            nc.vector.memset(v_sb[:], 1.0)

            # ---- phase-1 unit emitters ----
            def emit_qk(dst, w_sb, g):
                gsl = slice(g * QGS, (g + 1) * QGS)
                ps = mm_psum.tile([128, QGS], F32, tag="mmps")
                for kk in range(8):
                    nc.tensor.matmul(ps[:], w_sb[:, kk, :],
                                     xT_sb[:, kk, gsl],
                                     start=(kk == 0), stop=(kk == 7))
                nc.vector.tensor_copy(dst[:, gsl], ps[:])

            def emit_v(g):
                gsl = slice(g * QGS, (g + 1) * QGS)
                ps = mm_psum.tile([128, QGS], F32, tag="mmps")
                for kk in range(8):
                    nc.tensor.matmul(ps[:], wv_sb[:, kk, :],
                                     xT_sb[:, kk, gsl],
                                     start=(kk == 0), stop=(kk == 7))
                nc.vector.tensor_copy(vT_sb[:, gsl], ps[:])
                for kt in range(4 * g, 4 * g + 4):
                    tp = mm_psum.tile([128, QGS], F32, tag="mmps")
                    nc.tensor.transpose(tp[:, 0:128],
                                        vT_sb[:, kt * 128:(kt + 1) * 128],
                                        ident[:])
                    nc.vector.tensor_copy(v_sb[:, kt, 0:64], tp[:, 0:64])
                    nc.vector.tensor_copy(v_sb[:, kt, 65:129], tp[:, 64:128])

            def emit_proj(tt):
                tsl = slice(tt * 128, (tt + 1) * 128)
                ob = opool.tile([128, D], F32, tag="ob")
                for ngi in range(2):
                    nsl = slice(ngi * 512, (ngi + 1) * 512)
                    pj = mm_psum.tile([128, QGS], F32, tag="mmps")
                    nc.tensor.matmul(pj[:], yT_sb[:, tsl], wp_sb[:, nsl],
                                     start=True, stop=True)
                    nc.vector.tensor_copy(ob[:, nsl], pj[:])
                nc.sync.dma_start(out[tsl, :], ob[:])

            # slack work interleaved into the attention loop, keyed by
            # (group, pair); consumed just before that pair's score matmuls
            # NOTE: emission order IS the dependency order — every unit must
            # be emitted before the first instruction that reads its output
            # (sc(g,p) is emitted 2 pairs early, y(g,p) on time)
            slack = {
                (0, 0): [lambda: emit_v(0)],
                (0, 1): [lambda: emit_v(1)],
                (0, 2): [lambda: emit_qk(kT_sb, wk_sb, 2)],
                (0, 3): [lambda: emit_v(2)],
                (0, 4): [lambda: emit_qk(kT_sb, wk_sb, 3)],
                (0, 5): [lambda: emit_v(3)],
                (0, 6): [lambda: emit_qk(qT_sb, wq_sb, 1)],
                (1, 0): [lambda: emit_qk(qT_sb, wq_sb, 2)],
                (2, 0): [lambda: emit_qk(qT_sb, wq_sb, 3)],
            }
            for g in (1, 2, 3):
                for p in range(4):
                    slack[(g, p + 1)] = [
                        (lambda tt: lambda: emit_proj(tt))((g - 1) * 4 + p)]

            # ---- prologue: K groups 0-1 and Q for group 0 ----
            emit_qk(kT_sb, wk_sb, 0)
            emit_qk(qT_sb, wq_sb, 0)
            emit_qk(kT_sb, wk_sb, 1)

            def emit_sc(g, p):
                """Score matmuls + exp for k-tile pair p of group g."""
                gsl = slice(g * QGS, (g + 1) * QGS)
                sc0 = sc_psum.tile([128, 2, QGS], F32, tag="sc")
                sc1 = sc_psum.tile([128, 2, QGS], F32, tag="sc")
                for j in range(2):
                    ksl = slice((2 * p + j) * 128, (2 * p + j + 1) * 128)
                    nc.tensor.matmul(sc0[:, j, :], kT_sb[0:64, ksl],
                                     qT_sb[0:64, gsl], start=True,
                                     stop=True, tile_position=(0, 0))
                    nc.tensor.matmul(sc1[:, j, :], kT_sb[64:128, ksl],
                                     qT_sb[64:128, gsl], start=True,
                                     stop=True, tile_position=(64, 0))
                e0 = epool.tile([128, 2, QGS], BF16, tag="e")
                e1 = epool.tile([128, 2, QGS], BF16, tag="e")
                nc.scalar.activation(e0[:], sc0[:], Exp)
                nc.scalar.activation(e1[:], sc1[:], Exp)
                return e0, e1

            # ---- attention main loop ----
            for g in range(QG):
                gsl = slice(g * QGS, (g + 1) * QGS)
                y0 = y_psum.tile([65, QGS], F32, tag="y")
                y1 = y_psum.tile([65, QGS], F32, tag="y")
                pend = {0: emit_sc(g, 0), 1: emit_sc(g, 1)}
                for p in range(PAIRS):
                    for fn in slack.get((g, p), ()):
                        fn()
                    if p + 2 < PAIRS:
                        pend[p + 2] = emit_sc(g, p + 2)
                    e0, e1 = pend.pop(p)
                    for j in range(2):
                        kt = 2 * p + j
                        nc.tensor.matmul(y0[:], v_sb[:, kt, 0:65], e0[:, j, :],
                                         start=(kt == 0), stop=(kt == KT - 1))
                        nc.tensor.matmul(y1[:], v_sb[:, kt, 65:130], e1[:, j, :],
                                         start=(kt == 0), stop=(kt == KT - 1))

                # ---- group epilogue: normalize y, build yT ----
                sr0 = spool.tile([1, QGS], F32, tag="sr0")
                sr1 = spool.tile([1, QGS], F32, tag="sr1")
                rb0 = spool.tile([64, QGS], F32, tag="rb0")
                rb1 = spool.tile([64, QGS], F32, tag="rb1")
                tmp1 = spool.tile([64, QGS], BF16, tag="tmp1")
                nc.vector.reciprocal(sr0[:], y0[64:65, :])
                nc.vector.reciprocal(sr1[:], y1[64:65, :])
                nc.gpsimd.partition_broadcast(rb0[:], sr0[:])
                nc.gpsimd.partition_broadcast(rb1[:], sr1[:])
                nc.vector.scalar_tensor_tensor(
                    yT_sb[0:64, gsl], y0[0:64, :], 1.0, rb0[:],
                    op0=MULT, op1=MULT)
                nc.vector.scalar_tensor_tensor(
                    tmp1[:], y1[0:64, :], 1.0, rb1[:], op0=MULT, op1=MULT)
                nc.gpsimd.dma_start(yT_sb[64:128, gsl], tmp1[:])

            # ---- tail: last group's projection ----
            for tt in range(12, 16):
                emit_proj(tt)

    nc.compile()
    return nc


_nc_cache = None


def kernel(x: np.ndarray, W_qkv: np.ndarray, W_proj: np.ndarray) -> np.ndarray:
    global _nc_cache, last_results
    assert x.shape == (B, T, D)
    x2d = np.ascontiguousarray(x.reshape(T, D))
    xT = np.ascontiguousarray(x2d.T).astype(ml_dtypes.bfloat16)
    scale = 1.0 / np.sqrt(np.float32(HD))

    in_maps = []
    for dev in range(NCORES):
        wq = (W_qkv[:, dev * DL:(dev + 1) * DL] * scale).astype(
            ml_dtypes.bfloat16)
        wk = W_qkv[:, D + dev * DL: D + (dev + 1) * DL].astype(
            ml_dtypes.bfloat16)
        wv = W_qkv[:, 2 * D + dev * DL: 2 * D + (dev + 1) * DL].astype(
            ml_dtypes.bfloat16)
        wp = W_proj[dev * DL:(dev + 1) * DL, :].astype(ml_dtypes.bfloat16)
        in_maps.append({
            "xT": xT,
            "wq": np.ascontiguousarray(wq),
            "wk": np.ascontiguousarray(wk),
            "wv": np.ascontiguousarray(wv),
            "wp": np.ascontiguousarray(wp),
        })

    if _nc_cache is None:
        _nc_cache = build_nc()
    res = bass_utils.run_bass_kernel_spmd(
        _nc_cache, in_maps, core_ids=list(range(NCORES)),
        tmpdir=os.environ.get("BASS_KERNEL_TMPDIR"))
    last_results = res
    total = np.zeros((T, D), dtype=np.float64)
    for dev in range(NCORES):
        total += res.results[dev]["out"].astype(np.float64)
    return total.astype(np.float32).reshape(B, T, D)


if __name__ == "__main__":
    rng = np.random.default_rng(0)
    x = rng.standard_normal((B, T, D)).astype(np.float32)
    wqkv = (rng.standard_normal((D, 3 * D)) * 0.02).astype(np.float32)
    wproj = (rng.standard_normal((D, D)) * 0.02).astype(np.float32)
    y = kernel(x, wqkv, wproj)
    print("kernel output", y.shape, y.dtype, float(np.abs(y).mean()))


# revision 23
# speedup vs baseline: 1.1142x; 1.1142x over previous
"""Trainium2 Bass kernel for 16-head self-attention (B=1, T=2048, d=1024).

Sharding: 2 heads per NeuronCore (tensor-parallel over QKV columns / proj
rows) across 8 cores; each core emits a partial [T, d] projection output,
summed on the host.

Device-side dataflow (per core, heads h0/h1 on partition halves):
  qT/kT [128, T] bf16 (head-dim-major) from full-C chunked matmuls
  vT [128, T] f32 likewise, PE-transposed into per-k-tile [t, v|1] bf16 tiles
  scores S[k, q] per k-tile via row-tiled matmul pairs (both heads share
  the PE array concurrently); exp on ACT -> bf16 e-tiles [128, 2, 512]
  y[65, q] accumulated per k-tile (row 64 = softmax sums via ones column)
  y normalized pre-projection (fast reciprocal + partition_broadcast + STT)
  proj with C=128 (both heads contracted in one matmul); f32 out DMA
K/V/Q generation and the previous group's projection are interleaved into
the attention loop, and scores run two k-tile pairs ahead of the exp so
the scalar engine (exp) stays the pacing engine.
"""
import sys, os

sys.path.insert(0, "/opt/trn_rl_repo")

import numpy as np
import ml_dtypes

import concourse.bass as bass
import concourse.bacc as bacc
import concourse.tile as tile
from concourse import mybir
from concourse import bass_utils
from concourse.masks import make_identity

BF16 = mybir.dt.bfloat16
F32 = mybir.dt.float32

B, T, D = 1, 2048, 1024
H = 16
HD = D // H          # 64
NCORES = 8
HPD = H // NCORES    # 2 heads per device
DL = HPD * HD        # 128 local head dims per device
QG = 4               # q groups of 512
QGS = T // QG        # 512
KT = T // 128        # 16 k tiles
PAIRS = KT // 2      # 8 k-tile pairs per q group

last_results = None  # BassKernelResults of the most recent run (for test.py)

Exp = mybir.ActivationFunctionType.Exp
MULT = mybir.AluOpType.mult
DIV = mybir.AluOpType.divide


def build_nc():
    nc = bacc.Bacc("TRN2", target_bir_lowering=False, debug=False,
                   num_devices=NCORES)
    xT = nc.dram_tensor("xT", [D, T], BF16, kind="ExternalInput").ap()
    wq = nc.dram_tensor("wq", [D, DL], BF16, kind="ExternalInput").ap()
    wk = nc.dram_tensor("wk", [D, DL], BF16, kind="ExternalInput").ap()
    wv = nc.dram_tensor("wv", [D, DL], BF16, kind="ExternalInput").ap()
    wp = nc.dram_tensor("wp", [DL, D], BF16, kind="ExternalInput").ap()
    out = nc.dram_tensor("out", [T, D], F32, kind="ExternalOutput").ap()

    with tile.TileContext(nc) as tc:
        with (
            tc.tile_pool(name="const", bufs=1) as cpool,
            tc.tile_pool(name="work", bufs=2) as spool,
            tc.tile_pool(name="epool", bufs=6) as epool,
            tc.tile_pool(name="opool", bufs=2) as opool,
            tc.tile_pool(name="mm", bufs=2, space="PSUM") as mm_psum,
            tc.tile_pool(name="sc", bufs=2, space="PSUM") as sc_psum,
            tc.tile_pool(name="yp", bufs=2, space="PSUM") as y_psum,
        ):
            # ---- persistent SBUF tensors ----
            xT_sb = cpool.tile([128, 8, T], BF16)       # x^T, d-chunk-major
            wq_sb = cpool.tile([128, 8, DL], BF16)
            wk_sb = cpool.tile([128, 8, DL], BF16)
            wv_sb = cpool.tile([128, 8, DL], BF16)
            wp_sb = cpool.tile([128, D], BF16)          # both heads' proj rows
            ident = cpool.tile([128, 128], F32)
            qT_sb = cpool.tile([128, T], BF16)
            kT_sb = cpool.tile([128, T], BF16)
            vT_sb = cpool.tile([128, T], F32)           # v^T (pre-transpose)
            v_sb = cpool.tile([128, KT, 130], BF16)     # [t, kt, v0|1|v1|1]
            yT_sb = cpool.tile([128, T], BF16)          # normalized y^T

            # ---- input DMAs ----
            # weights as single batched transfers; xT chunked along d AND
            # halved along t so group-0 K/Q matmuls can start early
            xTr = xT.rearrange("(n p) t -> p n t", p=128)
            wqr = wq.rearrange("(n p) m -> p n m", p=128)
            wkr = wk.rearrange("(n p) m -> p n m", p=128)
            wvr = wv.rearrange("(n p) m -> p n m", p=128)
            # scalar (ACT) is a hardware DGE and idle here: it carries the
            # critical xT half-0 while sync/gpsimd push the weight chunks
            for kk in range(8):
                nc.scalar.dma_start(xT_sb[:, kk, 0:1024], xTr[:, kk, 0:1024])
            for kk in range(8):
                nc.sync.dma_start(wk_sb[:, kk, :], wkr[:, kk, :])
                nc.gpsimd.dma_start(wq_sb[:, kk, :], wqr[:, kk, :])
            for kk in range(8):
                eng = nc.sync if kk % 2 == 0 else nc.gpsimd
                eng.dma_start(xT_sb[:, kk, 1024:2048], xTr[:, kk, 1024:2048])
            for kk in range(8):
                nc.scalar.dma_start(wv_sb[:, kk, :], wvr[:, kk, :])
            nc.scalar.dma_start(wp_sb[:], wp[:, :])

            make_identity(nc, ident[:])
            nc.vector.memset(v_sb[:], 1.0)

            # ---- phase-1 unit emitters ----
            def emit_qk(dst, w_sb, g):
                gsl = slice(g * QGS, (g + 1) * QGS)
                ps = mm_psum.tile([128, QGS], F32, tag="mmps")
                for kk in range(8):
                    nc.tensor.matmul(ps[:], w_sb[:, kk, :],
                                     xT_sb[:, kk, gsl],
                                     start=(kk == 0), stop=(kk == 7))
                nc.vector.tensor_copy(dst[:, gsl], ps[:])

            def emit_v(g):
                gsl = slice(g * QGS, (g + 1) * QGS)
                ps = mm_psum.tile([128, QGS], F32, tag="mmps")
                for kk in range(8):
                    nc.tensor.matmul(ps[:], wv_sb[:, kk, :],
                                     xT_sb[:, kk, gsl],
                                     start=(kk == 0), stop=(kk == 7))
                nc.vector.tensor_copy(vT_sb[:, gsl], ps[:])
                for kt in range(4 * g, 4 * g + 4):
                    tp = mm_psum.tile([128, QGS], F32, tag="mmps")
                    nc.tensor.transpose(tp[:, 0:128],
                                        vT_sb[:, kt * 128:(kt + 1) * 128],
                                        ident[:])
                    nc.vector.tensor_copy(v_sb[:, kt, 0:64], tp[:, 0:64])
                    nc.vector.tensor_copy(v_sb[:, kt, 65:129], tp[:, 64:128])

            def emit_proj(tt):
                tsl = slice(tt * 128, (tt + 1) * 128)
                ob = opool.tile([128, D], F32, tag="ob")
                for ngi in range(2):
                    nsl = slice(ngi * 512, (ngi + 1) * 512)
                    pj = mm_psum.tile([128, QGS], F32, tag="mmps")
                    nc.tensor.matmul(pj[:], yT_sb[:, tsl], wp_sb[:, nsl],
                                     start=True, stop=True)
                    nc.vector.tensor_copy(ob[:, nsl], pj[:])
                    nc.sync.dma_start(out[tsl, nsl], ob[:, nsl])

            # slack work interleaved into the attention loop, keyed by
            # (group, pair); consumed just before that pair's score matmuls
            # NOTE: emission order IS the dependency order — every unit must
            # be emitted before the first instruction that reads its output
            # (sc(g,p) is emitted 2 pairs early, y(g,p) on time)
            slack = {
                (0, 0): [lambda: emit_v(0)],
                (0, 1): [lambda: emit_v(1)],
                (0, 2): [lambda: emit_qk(kT_sb, wk_sb, 2)],
                (0, 3): [lambda: emit_v(2)],
                (0, 4): [lambda: emit_qk(kT_sb, wk_sb, 3)],
                (0, 5): [lambda: emit_v(3)],
                (0, 6): [lambda: emit_qk(qT_sb, wq_sb, 1)],
                (1, 0): [lambda: emit_qk(qT_sb, wq_sb, 2)],
                (2, 0): [lambda: emit_qk(qT_sb, wq_sb, 3)],
            }
            for g in (1, 2, 3):
                off = 1 if g < 3 else 0
                for p in range(4):
                    slack[(g, p + off)] = slack.get((g, p + off), []) + [
                        (lambda tt: lambda: emit_proj(tt))((g - 1) * 4 + p)]

            # ---- prologue: K groups 0-1 and Q for group 0 ----
            emit_qk(kT_sb, wk_sb, 0)
            emit_qk(qT_sb, wq_sb, 0)
            emit_qk(kT_sb, wk_sb, 1)

            def emit_sc(g, p):
                """Score matmuls + exp for k-tile pair p of group g."""
                gsl = slice(g * QGS, (g + 1) * QGS)
                sc0 = sc_psum.tile([128, 2, QGS], F32, tag="sc")
                sc1 = sc_psum.tile([128, 2, QGS], F32, tag="sc")
                for j in range(2):
                    ksl = slice((2 * p + j) * 128, (2 * p + j + 1) * 128)
                    nc.tensor.matmul(sc0[:, j, :], kT_sb[0:64, ksl],
                                     qT_sb[0:64, gsl], start=True,
                                     stop=True, tile_position=(0, 0))
                    nc.tensor.matmul(sc1[:, j, :], kT_sb[64:128, ksl],
                                     qT_sb[64:128, gsl], start=True,
                                     stop=True, tile_position=(64, 0))
                e0 = epool.tile([128, 2, QGS], BF16, tag="e")
                e1 = epool.tile([128, 2, QGS], BF16, tag="e")
                nc.scalar.activation(e0[:], sc0[:], Exp)
                nc.scalar.activation(e1[:], sc1[:], Exp)
                return e0, e1

            # ---- attention main loop ----
            for g in range(QG):
                gsl = slice(g * QGS, (g + 1) * QGS)
                y0 = y_psum.tile([65, QGS], F32, tag="y")
                y1 = y_psum.tile([65, QGS], F32, tag="y")
                pend = {0: emit_sc(g, 0), 1: emit_sc(g, 1)}
                for p in range(PAIRS):
                    for fn in slack.get((g, p), ()):
                        fn()
                    if p + 2 < PAIRS:
                        pend[p + 2] = emit_sc(g, p + 2)
                    e0, e1 = pend.pop(p)
                    for j in range(2):
                        kt = 2 * p + j
                        nc.tensor.matmul(y0[:], v_sb[:, kt, 0:65], e0[:, j, :],
                                         start=(kt == 0), stop=(kt == KT - 1))
                        nc.tensor.matmul(y1[:], v_sb[:, kt, 65:130], e1[:, j, :],
                                         start=(kt == 0), stop=(kt == KT - 1))

                # ---- group epilogue: normalize y, build yT ----
                sr0 = spool.tile([1, QGS], F32, tag="sr0")
                sr1 = spool.tile([1, QGS], F32, tag="sr1")
                rb0 = spool.tile([64, QGS], F32, tag="rb0")
                rb1 = spool.tile([64, QGS], F32, tag="rb1")
                tmp1 = spool.tile([64, QGS], BF16, tag="tmp1")
                ss0 = spool.tile([1, QGS], F32, tag="ss0")
                ss1 = spool.tile([1, QGS], F32, tag="ss1")
                nc.vector.tensor_copy(ss0[:], y0[64:65, :])
                nc.vector.tensor_copy(ss1[:], y1[64:65, :])
                nc.vector.reciprocal_approx_fast(sr0[:], ss0[:])
                nc.vector.reciprocal_approx_fast(sr1[:], ss1[:])
                nc.gpsimd.partition_broadcast(rb0[:], sr0[:])
                nc.gpsimd.partition_broadcast(rb1[:], sr1[:])
                nc.vector.scalar_tensor_tensor(
                    yT_sb[0:64, gsl], y0[0:64, :], 1.0, rb0[:],
                    op0=MULT, op1=MULT)
                nc.vector.scalar_tensor_tensor(
                    tmp1[:], y1[0:64, :], 1.0, rb1[:], op0=MULT, op1=MULT)
                nc.gpsimd.dma_start(yT_sb[64:128, gsl], tmp1[:])

            # ---- tail: last group's projection ----
            for tt in range(12, 16):
                emit_proj(tt)

    nc.compile()
    return nc


_nc_cache = None


def kernel(x: np.ndarray, W_qkv: np.ndarray, W_proj: np.ndarray) -> np.ndarray:
    global _nc_cache, last_results
    assert x.shape == (B, T, D)
    x2d = np.ascontiguousarray(x.reshape(T, D))
    xT = np.ascontiguousarray(x2d.T).astype(ml_dtypes.bfloat16)
    scale = 1.0 / np.sqrt(np.float32(HD))

    in_maps = []
    for dev in range(NCORES):
        wq = (W_qkv[:, dev * DL:(dev + 1) * DL] * scale).astype(
            ml_dtypes.bfloat16)
        wk = W_qkv[:, D + dev * DL: D + (dev + 1) * DL].astype(
            ml_dtypes.bfloat16)
        wv = W_qkv[:, 2 * D + dev * DL: 2 * D + (dev + 1) * DL].astype(
            ml_dtypes.bfloat16)
        wp = W_proj[dev * DL:(dev + 1) * DL, :].astype(ml_dtypes.bfloat16)
        in_maps.append({
            "xT": xT,
            "wq": np.ascontiguousarray(wq),
            "wk": np.ascontiguousarray(wk),
            "wv": np.ascontiguousarray(wv),
            "wp": np.ascontiguousarray(wp),
        })

    if _nc_cache is None:
        _nc_cache = build_nc()
    res = bass_utils.run_bass_kernel_spmd(
        _nc_cache, in_maps, core_ids=list(range(NCORES)),
        tmpdir=os.environ.get("BASS_KERNEL_TMPDIR"))
    last_results = res
    total = np.zeros((T, D), dtype=np.float64)
    for dev in range(NCORES):
        total += res.results[dev]["out"].astype(np.float64)
    return total.astype(np.float32).reshape(B, T, D)


if __name__ == "__main__":
    rng = np.random.default_rng(0)
    x = rng.standard_normal((B, T, D)).astype(np.float32)
    wqkv = (rng.standard_normal((D, 3 * D)) * 0.02).astype(np.float32)
    wproj = (rng.standard_normal((D, D)) * 0.02).astype(np.float32)
    y = kernel(x, wqkv, wproj)
    print("kernel output", y.shape, y.dtype, float(np.abs(y).mean()))


# revision 24
# speedup vs baseline: 1.2176x; 1.0928x over previous
"""Trainium2 Bass kernel for 16-head self-attention (B=1, T=2048, d=1024).

Sharding: 2 heads per NeuronCore (tensor-parallel over QKV columns / proj
rows) across 8 cores; each core emits a partial [T, d] projection output,
summed on the host.

Device-side dataflow (per core, heads h0/h1 on partition halves):
  qT/kT [128, T] bf16 (head-dim-major) from full-C chunked matmuls
  vT [128, T] f32 likewise, PE-transposed into per-k-tile [t, v|1] bf16 tiles
  scores S[k, q] per k-tile via row-tiled matmul pairs (both heads share
  the PE array concurrently); exp on ACT -> bf16 e-tiles [128, 2, 512]
  y[65, q] accumulated per k-tile (row 64 = softmax sums via ones column)
  y normalized pre-projection (fast reciprocal + partition_broadcast + STT)
  proj with C=128 (both heads contracted in one matmul); f32 out DMA
K/V/Q generation and the previous group's projection are interleaved into
the attention loop, and scores run two k-tile pairs ahead of the exp so
the scalar engine (exp) stays the pacing engine.
"""
import sys, os

sys.path.insert(0, "/opt/trn_rl_repo")

import numpy as np
import ml_dtypes

import concourse.bass as bass
import concourse.bacc as bacc
import concourse.tile as tile
from concourse import mybir
from concourse import bass_utils
from concourse.masks import make_identity

BF16 = mybir.dt.bfloat16
F32 = mybir.dt.float32

B, T, D = 1, 2048, 1024
H = 16
HD = D // H          # 64
NCORES = 8
HPD = H // NCORES    # 2 heads per device
DL = HPD * HD        # 128 local head dims per device
QG = 4               # q groups of 512
QGS = T // QG        # 512
KT = T // 128        # 16 k tiles
PAIRS = KT // 2      # 8 k-tile pairs per q group

last_results = None  # BassKernelResults of the most recent run (for test.py)

Exp = mybir.ActivationFunctionType.Exp
MULT = mybir.AluOpType.mult
DIV = mybir.AluOpType.divide


def build_nc():
    nc = bacc.Bacc("TRN2", target_bir_lowering=False, debug=False,
                   num_devices=NCORES)
    xT = nc.dram_tensor("xT", [D, T], BF16, kind="ExternalInput").ap()
    wq = nc.dram_tensor("wq", [D, DL], BF16, kind="ExternalInput").ap()
    wk = nc.dram_tensor("wk", [D, DL], BF16, kind="ExternalInput").ap()
    wv = nc.dram_tensor("wv", [D, DL], BF16, kind="ExternalInput").ap()
    wp = nc.dram_tensor("wp", [DL, D], BF16, kind="ExternalInput").ap()
    out = nc.dram_tensor("out", [T, D], F32, kind="ExternalOutput").ap()

    with tile.TileContext(nc) as tc:
        with (
            tc.tile_pool(name="const", bufs=1) as cpool,
            tc.tile_pool(name="work", bufs=2) as spool,
            tc.tile_pool(name="epool", bufs=6) as epool,
            tc.tile_pool(name="opool", bufs=2) as opool,
            tc.tile_pool(name="mm", bufs=2, space="PSUM") as mm_psum,
            tc.tile_pool(name="sc", bufs=2, space="PSUM") as sc_psum,
            tc.tile_pool(name="yp", bufs=2, space="PSUM") as y_psum,
        ):
            # ---- persistent SBUF tensors ----
            xT_sb = cpool.tile([128, 8, T], BF16)       # x^T, d-chunk-major
            wq_sb = cpool.tile([128, 8, DL], BF16)
            wk_sb = cpool.tile([128, 8, DL], BF16)
            wv_sb = cpool.tile([128, 8, DL], BF16)
            wp_sb = cpool.tile([128, D], BF16)          # both heads' proj rows
            ident = cpool.tile([128, 128], F32)
            qT_sb = cpool.tile([128, T], BF16)
            kT_sb = cpool.tile([128, T], BF16)
            vT_sb = cpool.tile([128, T], F32)           # v^T (pre-transpose)
            v_sb = cpool.tile([128, KT, 130], BF16)     # [t, kt, v0|1|v1|1]
            yT_sb = cpool.tile([128, T], BF16)          # normalized y^T

            # ---- input DMAs ----
            # weights as single batched transfers; xT chunked along d AND
            # halved along t so group-0 K/Q matmuls can start early
            xTr = xT.rearrange("(n p) t -> p n t", p=128)
            wqr = wq.rearrange("(n p) m -> p n m", p=128)
            wkr = wk.rearrange("(n p) m -> p n m", p=128)
            wvr = wv.rearrange("(n p) m -> p n m", p=128)
            # scalar (ACT) is a hardware DGE and idle here: it carries the
            # critical xT half-0 while gpsimd pushes the weight chunks;
            # sync's queue starts ~7us late (semaphore setup), so it only
            # gets the late half-1 chunks
            for kk in range(8):
                nc.scalar.dma_start(xT_sb[:, kk, 0:1024], xTr[:, kk, 0:1024])
            for kk in range(8):
                nc.gpsimd.dma_start(wk_sb[:, kk, :], wkr[:, kk, :])
                nc.gpsimd.dma_start(wq_sb[:, kk, :], wqr[:, kk, :])
            for kk in range(8):
                eng = nc.sync if kk % 2 == 0 else nc.scalar
                eng.dma_start(xT_sb[:, kk, 1024:2048], xTr[:, kk, 1024:2048])
            for kk in range(8):
                nc.sync.dma_start(wv_sb[:, kk, :], wvr[:, kk, :])
            nc.sync.dma_start(wp_sb[:], wp[:, :])

            make_identity(nc, ident[:])
            nc.vector.memset(v_sb[:], 1.0)

            # ---- phase-1 unit emitters ----
            def emit_qk(dst, w_sb, g):
                gsl = slice(g * QGS, (g + 1) * QGS)
                ps = mm_psum.tile([128, QGS], F32, tag="mmps")
                for kk in range(8):
                    nc.tensor.matmul(ps[:], w_sb[:, kk, :],
                                     xT_sb[:, kk, gsl],
                                     start=(kk == 0), stop=(kk == 7))
                nc.vector.tensor_copy(dst[:, gsl], ps[:])

            def emit_v(g):
                gsl = slice(g * QGS, (g + 1) * QGS)
                ps = mm_psum.tile([128, QGS], F32, tag="mmps")
                for kk in range(8):
                    nc.tensor.matmul(ps[:], wv_sb[:, kk, :],
                                     xT_sb[:, kk, gsl],
                                     start=(kk == 0), stop=(kk == 7))
                nc.vector.tensor_copy(vT_sb[:, gsl], ps[:])
                for kt in range(4 * g, 4 * g + 4):
                    tp = mm_psum.tile([128, QGS], F32, tag="mmps")
                    nc.tensor.transpose(tp[:, 0:128],
                                        vT_sb[:, kt * 128:(kt + 1) * 128],
                                        ident[:])
                    nc.vector.tensor_copy(v_sb[:, kt, 0:64], tp[:, 0:64])
                    nc.vector.tensor_copy(v_sb[:, kt, 65:129], tp[:, 64:128])

            def emit_proj(tt, tail=False):
                tsl = slice(tt * 128, (tt + 1) * 128)
                ob = opool.tile([128, D], F32, tag="ob")
                for ngi in range(2):
                    nsl = slice(ngi * 512, (ngi + 1) * 512)
                    pj = mm_psum.tile([128, QGS], F32, tag="mmps")
                    nc.tensor.matmul(pj[:], yT_sb[:, tsl], wp_sb[:, nsl],
                                     start=True, stop=True)
                    if tail and ngi == 0:
                        nc.scalar.copy(ob[:, nsl], pj[:])
                    else:
                        nc.vector.tensor_copy(ob[:, nsl], pj[:])
                    nc.sync.dma_start(out[tsl, nsl], ob[:, nsl])

            # slack work interleaved into the attention loop, keyed by
            # (group, pair); consumed just before that pair's score matmuls
            # NOTE: emission order IS the dependency order — every unit must
            # be emitted before the first instruction that reads its output
            # (sc(g,p) is emitted 2 pairs early, y(g,p) on time)
            slack = {
                (0, 0): [lambda: emit_qk(kT_sb, wk_sb, 1), lambda: emit_v(0)],
                (0, 1): [lambda: emit_v(1)],
                (0, 2): [lambda: emit_qk(kT_sb, wk_sb, 2)],
                (0, 3): [lambda: emit_v(2)],
                (0, 4): [lambda: emit_qk(kT_sb, wk_sb, 3)],
                (0, 5): [lambda: emit_v(3)],
                (0, 6): [lambda: emit_qk(qT_sb, wq_sb, 1)],
                (1, 0): [lambda: emit_qk(qT_sb, wq_sb, 2)],
                (2, 0): [lambda: emit_qk(qT_sb, wq_sb, 3)],
            }
            for g in (1, 2, 3):
                off = 1 if g < 3 else 0
                for p in range(4):
                    slack[(g, p + off)] = slack.get((g, p + off), []) + [
                        (lambda tt: lambda: emit_proj(tt))((g - 1) * 4 + p)]

            # ---- prologue: K and Q for group 0 ----
            emit_qk(kT_sb, wk_sb, 0)
            emit_qk(qT_sb, wq_sb, 0)

            def emit_sc(g, p):
                """Score matmuls + exp for k-tile pair p of group g."""
                gsl = slice(g * QGS, (g + 1) * QGS)
                sc0 = sc_psum.tile([128, 2, QGS], F32, tag="sc")
                sc1 = sc_psum.tile([128, 2, QGS], F32, tag="sc")
                for j in range(2):
                    ksl = slice((2 * p + j) * 128, (2 * p + j + 1) * 128)
                    nc.tensor.matmul(sc0[:, j, :], kT_sb[0:64, ksl],
                                     qT_sb[0:64, gsl], start=True,
                                     stop=True, tile_position=(0, 0))
                    nc.tensor.matmul(sc1[:, j, :], kT_sb[64:128, ksl],
                                     qT_sb[64:128, gsl], start=True,
                                     stop=True, tile_position=(64, 0))
                e0 = epool.tile([128, 2, QGS], BF16, tag="e")
                e1 = epool.tile([128, 2, QGS], BF16, tag="e")
                nc.scalar.activation(e0[:], sc0[:], Exp)
                nc.scalar.activation(e1[:], sc1[:], Exp)
                return e0, e1

            # ---- attention main loop ----
            for g in range(QG):
                gsl = slice(g * QGS, (g + 1) * QGS)
                y0 = y_psum.tile([65, QGS], F32, tag="y")
                y1 = y_psum.tile([65, QGS], F32, tag="y")
                pend = {0: emit_sc(g, 0), 1: emit_sc(g, 1)}
                for p in range(PAIRS):
                    for fn in slack.get((g, p), ()):
                        fn()
                    if p + 2 < PAIRS:
                        pend[p + 2] = emit_sc(g, p + 2)
                    e0, e1 = pend.pop(p)
                    for j in range(2):
                        kt = 2 * p + j
                        nc.tensor.matmul(y0[:], v_sb[:, kt, 0:65], e0[:, j, :],
                                         start=(kt == 0), stop=(kt == KT - 1))
                        nc.tensor.matmul(y1[:], v_sb[:, kt, 65:130], e1[:, j, :],
                                         start=(kt == 0), stop=(kt == KT - 1))

                # ---- group epilogue: normalize y, build yT ----
                sr0 = spool.tile([1, QGS], F32, tag="sr0")
                sr1 = spool.tile([1, QGS], F32, tag="sr1")
                rb0 = spool.tile([64, QGS], F32, tag="rb0")
                rb1 = spool.tile([64, QGS], F32, tag="rb1")
                tmp1 = spool.tile([64, QGS], BF16, tag="tmp1")
                ss0 = spool.tile([1, QGS], F32, tag="ss0")
                ss1 = spool.tile([1, QGS], F32, tag="ss1")
                nc.vector.tensor_copy(ss0[:], y0[64:65, :])
                nc.vector.tensor_copy(ss1[:], y1[64:65, :])
                nc.vector.reciprocal_approx_fast(sr0[:], ss0[:])
                nc.vector.reciprocal_approx_fast(sr1[:], ss1[:])
                nc.gpsimd.partition_broadcast(rb0[:], sr0[:])
                nc.gpsimd.partition_broadcast(rb1[:], sr1[:])
                nc.vector.scalar_tensor_tensor(
                    yT_sb[0:64, gsl], y0[0:64, :], 1.0, rb0[:],
                    op0=MULT, op1=MULT)
                nc.vector.scalar_tensor_tensor(
                    tmp1[:], y1[0:64, :], 1.0, rb1[:], op0=MULT, op1=MULT)
                nc.gpsimd.dma_start(yT_sb[64:128, gsl], tmp1[:])

            # ---- tail: last group's projection ----
            for tt in range(12, 16):
                emit_proj(tt, tail=True)

    nc.compile()
    return nc


_nc_cache = None


def kernel(x: np.ndarray, W_qkv: np.ndarray, W_proj: np.ndarray) -> np.ndarray:
    global _nc_cache, last_results
    assert x.shape == (B, T, D)
    x2d = np.ascontiguousarray(x.reshape(T, D))
    xT = np.ascontiguousarray(x2d.T).astype(ml_dtypes.bfloat16)
    scale = 1.0 / np.sqrt(np.float32(HD))

    in_maps = []
    for dev in range(NCORES):
        wq = (W_qkv[:, dev * DL:(dev + 1) * DL] * scale).astype(
            ml_dtypes.bfloat16)
        wk = W_qkv[:, D + dev * DL: D + (dev + 1) * DL].astype(
            ml_dtypes.bfloat16)
        wv = W_qkv[:, 2 * D + dev * DL: 2 * D + (dev + 1) * DL].astype(
            ml_dtypes.bfloat16)
        wp = W_proj[dev * DL:(dev + 1) * DL, :].astype(ml_dtypes.bfloat16)
        in_maps.append({
            "xT": xT,
            "wq": np.ascontiguousarray(wq),
            "wk": np.ascontiguousarray(wk),
            "wv": np.ascontiguousarray(wv),
            "wp": np.ascontiguousarray(wp),
        })

    if _nc_cache is None:
        _nc_cache = build_nc()
    res = bass_utils.run_bass_kernel_spmd(
        _nc_cache, in_maps, core_ids=list(range(NCORES)),
        tmpdir=os.environ.get("BASS_KERNEL_TMPDIR"))
    last_results = res
    total = np.zeros((T, D), dtype=np.float64)
    for dev in range(NCORES):
        total += res.results[dev]["out"].astype(np.float64)
    return total.astype(np.float32).reshape(B, T, D)


if __name__ == "__main__":
    rng = np.random.default_rng(0)
    x = rng.standard_normal((B, T, D)).astype(np.float32)
    wqkv = (rng.standard_normal((D, 3 * D)) * 0.02).astype(np.float32)
    wproj = (rng.standard_normal((D, D)) * 0.02).astype(np.float32)
    y = kernel(x, wqkv, wproj)
    print("kernel output", y.shape, y.dtype, float(np.abs(y).mean()))
